# revision 2
# baseline (speedup 1.0000x reference)
"""Lorentz multi-head attention on 8 Trainium2 NeuronCores (v3, fp8).

Same head-parallel structure as v2 (see git history of this file): core c
computes head c for all batches; per-half-batch AllToAll redistributes
head-blocks to token-blocks; phase 2 (concat_logradius fusion + output
LorentzFC) is woven between the attention halves of the next batch.

v3 moves all phase-1 matmuls to fp8e4m3 DoubleRow (0.5 PE cycles/row):
- host pre-scales Wq/Wk/Wv by 8 (Lorentz midpoint normalization is
  invariant to a common q/k/v scale) so weight values clear the e4m3
  denormal range; the exp scale becomes -SCALE/64 and the t-row
  sqrt bias 1/K becomes 64/K.
- x is fed as fp8 [640, BN] (bias lane 513=1), weights as pre-paired
  k-tiles [128, 3, 2, M]; projections run 3 DoubleRow matmuls per
  512-token chunk (k-tile pair 2 is zero-padded via a once-memset x
  slice).
- q/k land in bf16 [65, N] (squares for the t-rows read them at DVE 4x),
  then a gpsimd cast makes an fp8 copy and two SBUF->SBUF DMAs fold it
  into the DoubleRow layout [33, 2, N] (dims 0-31 + zero pad | dims
  32-63 + t row). Scores = one K=33 DoubleRow matmul per (key-block,
  512 queries).
- exp writes fp8 directly into key-block PAIR tiles [128, 2, 1024]; AV
  consumes a pair per DoubleRow matmul (va pitched [128, 16, 80] since
  dual-fp8 ldweights needs 16B-aligned outer strides).

Midpoint drain, AllToAll, and phase 2 are unchanged from v2 (the
exchanged midpoints are scale-free after r-normalization).
"""

import os
import sys

sys.path.insert(0, "/opt/trn_rl_repo")

import numpy as np
import ml_dtypes

import concourse.bass as bass
import concourse.mybir as mybir
import concourse.tile as tile
from concourse import bacc, bass_utils
from concourse.masks import make_identity

# Problem constants (hardcoded per task contract)
B, N, D = 4, 2048, 513
H, DHS = 8, 64
NCORES = 8
KCURV = 0.1
INVK = 10.0
SCALE = 1.0 / np.sqrt(DHS)  # 0.125
S_CONST = 2.8479428291320801  # exp(0.5*(digamma(256)-digamma(32)))
W8 = 8.0  # host weight pre-scale (folded out by midpoint normalization)
DPAD = 640  # 513 padded to 5*128 (col 513 = constant-1 bias lane)
KC = 5  # contraction chunks of 128
BN = B * N  # 8192 tokens
RPC = BN // NCORES  # 1024 rows per core in phase 2 (256 per batch)
TPB = N // NCORES  # 256 tokens per core per batch
HTOK = TPB // 2  # 128 tokens per core per half-batch A2A
HALF = 1024  # query columns per attention half
F32 = mybir.dt.float32
BF16 = mybir.dt.bfloat16
FP8 = mybir.dt.float8e4
Ln = mybir.ActivationFunctionType.Ln
Exp = mybir.ActivationFunctionType.Exp
DR = mybir.MatmulPerfMode.DoubleRow

_CACHE = {}
BF = ml_dtypes.bfloat16
F8 = ml_dtypes.float8_e4m3


def _patch_act_tables(nc):
    # Exp and Ln both live in the natural_log_exp_and_others set; the
    # table-load pass picks the first set containing each function, which
    # splits them across two sets and reloads tables on every Ln<->Exp
    # switch (~1.3us each). Restrict the map so the combined set wins.
    from concourse.hw_specs import get_activation_tables

    try:
        tabs = get_activation_tables(nc.m.arch)
    except Exception:
        return
    if "natural_log_exp_and_others" not in tabs:
        return
    for name, fns in tabs.items():
        if name != "natural_log_exp_and_others":
            fns.discard(Exp)
            fns.discard(Ln)


def _build():
    nc = bacc.Bacc(
        "TRN2", target_bir_lowering=False, debug=False, num_devices=NCORES
    )
    _patch_act_tables(nc)

    x8_ap = nc.dram_tensor("x8", [DPAD, BN], FP8, kind="ExternalInput").ap()
    wqk_ap = nc.dram_tensor("wqk8", [128, 3, 2, 128], FP8,
                            kind="ExternalInput").ap()
    wv_ap = nc.dram_tensor("wv8", [128, 3, 2, DHS], FP8,
                           kind="ExternalInput").ap()
    woT_ap = nc.dram_tensor("woT", [DPAD, D - 1], BF16,
                            kind="ExternalInput").ap()
    y_ap = nc.dram_tensor("y", [RPC, D], F32, kind="ExternalOutput").ap()

    with tile.TileContext(nc) as tc:
        with (
            tc.tile_pool(name="const", bufs=1) as constp,
            tc.tile_pool(name="w", bufs=1) as wp,
            tc.tile_pool(name="x", bufs=1) as xp,
            tc.tile_pool(name="qk", bufs=1) as qkp,
            tc.tile_pool(name="att", bufs=1) as atp,
            tc.tile_pool(name="sm", bufs=1) as smp,
            tc.tile_pool(name="p2", bufs=1) as d2p,
            tc.tile_pool(name="ps", bufs=2, space="PSUM") as psp,
            tc.tile_pool(name="sc", bufs=2, space="PSUM") as scp,
            tc.tile_pool(name="mt", bufs=1, space="PSUM") as mtp,
            tc.tile_pool(name="dram", bufs=1, space="DRAM") as dramp,
        ):
            identB = constp.tile([128, 128], BF16)
            make_identity(nc, identB[:])
            signv = constp.tile([65, 1], BF16)
            nc.vector.memset(signv[0:64, :], -1.0)
            nc.vector.memset(signv[64:65, :], 1.0)
            # col 0 selects q rows (0-63), col 32 selects k rows (64-127):
            # activation-engine reads must start at partition 0/32/64, so
            # the k t-sum row lands on partition 32
            selqk = constp.tile([128, 33], BF16)
            nc.vector.memset(selqk[:], 0.0)
            nc.vector.memset(selqk[0:64, 0:1], 1.0)
            nc.vector.memset(selqk[64:128, 32:33], 1.0)
            onesv = constp.tile([64, 1], BF16)
            nc.vector.memset(onesv[:], 1.0)
            # t-row bias: 64/K because q/k/v are host-scaled by 8
            bias640 = constp.tile([128, 1], F32)
            nc.vector.memset(bias640[:], INVK * W8 * W8)
            # phase-2 biases use the unscaled 1/K (exchange is scale-free)
            bias10 = constp.tile([128, 1], F32)
            nc.vector.memset(bias10[:], INVK)
            biasD = constp.tile([128, 1], F32)
            nc.vector.memset(biasD[:], INVK * (1.0 + H * S_CONST * S_CONST))

            # Weights: pre-paired DoubleRow k-tiles (host-built fp8)
            wqkb = wp.tile([128, 3, 2, 128], FP8)
            wvb = wp.tile([128, 3, 2, DHS], FP8)
            wob = wp.tile([128, KC, D - 1], BF16)
            nc.sync.dma_start(wqkb[:], wqk_ap)
            nc.sync.dma_start(wvb[:], wv_ap)
            nc.sync.dma_start(wob[:], woT_ap.rearrange("(k p) s -> p k s",
                                                       p=128))

            # x slots: [128, 6, 2048] fp8, ki=5 stays zero (DoubleRow pad)
            x8s = []
            for i in range(3):
                t = xp.tile([128, 6, N], FP8, name=f"x8s{i}")
                nc.vector.memset(t[:, 5, :], 0.0)
                x8s.append(t)

            # fold slots: q~/k~ DoubleRow layout [33, 2, N]
            # tile0 = dims 0-31 + zero pad row, tile1 = dims 32-63 + t row
            qfolds, kfolds = [], []
            for i in range(2):
                qf = qkp.tile([33, 2, N], FP8, name=f"qfold{i}")
                kf = qkp.tile([33, 2, N], FP8, name=f"kfold{i}")
                nc.vector.memset(qf[32:33, 0, :], 0.0)
                nc.vector.memset(kf[32:33, 0, :], 0.0)
                qfolds.append(qf)
                kfolds.append(kf)

            sends = []
            recvs = []
            for b in range(B):
                sends.append(dramp.tile([N, DHS + 1], BF16, tag=f"send{b}",
                                        name=f"send{b}"))
                recvs.append([
                    dramp.tile([NCORES, HTOK, DHS + 1], BF16,
                               tag=f"recv{b}_{h}", name=f"recv{b}_{h}")
                    for h in range(2)
                ])

            qkv = {}

            # x loads are issued well ahead of each batch so they never
            # queue behind AllToAll traffic on the DMA engines
            def xload(b):
                xt = x8s[b % 3]
                if b != 0:
                    for ki in range(KC):
                        nc.sync.dma_start(
                            xt[:, ki, :],
                            x8_ap[ki * 128:(ki + 1) * 128, b * N:(b + 1) * N],
                        )
                else:
                    # nj-major column-split loads: proj(0) nj=0 needs only
                    # the first 512 columns of each contraction chunk, so
                    # those land first and compute starts early
                    for nj in range(4):
                        js = slice(nj * 512, (nj + 1) * 512)
                        for ki in range(KC):
                            nc.sync.dma_start(
                                xt[:, ki, js],
                                x8_ap[ki * 128:(ki + 1) * 128,
                                      nj * 512:(nj + 1) * 512],
                            )

            # ---- projections (q,k fused; v transposed) + t rows ----
            def proj(b):
                xt = x8s[b % 3]

                qa = qkp.tile([65, N], BF16, tag="qa", bufs=2, name=f"qa{b}")
                ka = qkp.tile([65, N], BF16, tag="ka", bufs=2, name=f"ka{b}")
                vT = qkp.tile([65, N], BF16, tag="vT", bufs=2, name=f"vT{b}")
                # row 0 = q sums, row 32 = k sums, row 64 = v sums
                # (partition-aligned for activation reads; rest is junk)
                tsta = smp.tile([65, N], F32, tag="tsta", bufs=2,
                                name=f"tsta{b}")
                for nj in range(N // 512):
                    js = slice(nj * 512, (nj + 1) * 512)
                    psqk = psp.tile([128, 512], F32, tag="ps",
                                    name=f"pqk{b}_{nj}")
                    for p in range(3):
                        nc.tensor.matmul(
                            psqk[:], wqkb[:, p, :, :],
                            xt[:, 2 * p:2 * p + 2, js],
                            start=(p == 0), stop=(p == 2), perf_mode=DR,
                        )
                    nc.vector.tensor_copy(qa[0:64, js], psqk[0:64, :])
                    nc.vector.tensor_copy(ka[0:64, js], psqk[64:128, :])
                    sqqk = smp.tile([128, 512], BF16, tag="sqqk", bufs=2,
                                    name=f"sqqk{b}_{nj}")
                    nc.vector.tensor_mul(sqqk[0:64, :], qa[0:64, js],
                                         qa[0:64, js])
                    nc.vector.tensor_mul(sqqk[64:128, :], ka[0:64, js],
                                         ka[0:64, js])
                    psv = psp.tile([64, 512], F32, tag="ps",
                                   name=f"pv{b}_{nj}")
                    for p in range(3):
                        nc.tensor.matmul(
                            psv[:], wvb[:, p, :, :],
                            xt[:, 2 * p:2 * p + 2, js],
                            start=(p == 0), stop=(p == 2), perf_mode=DR,
                        )
                    nc.vector.tensor_copy(vT[0:64, js], psv[:])
                    sqv = smp.tile([64, 512], BF16, tag="sqv", bufs=2,
                                   name=f"sqv{b}_{nj}")
                    nc.vector.tensor_mul(sqv[:], vT[0:64, js], vT[0:64, js])
                    ptr = psp.tile([65, 512], F32, tag="ps",
                                   name=f"ptr{b}_{nj}")
                    nc.tensor.matmul(ptr[0:33, :], selqk[:], sqqk[:],
                                     start=True, stop=True)
                    nc.tensor.matmul(ptr[64:65, :], onesv[:], sqv[:],
                                     start=True, stop=True)
                    nc.vector.tensor_copy(tsta[0:33, js], ptr[0:33, :])
                    nc.vector.tensor_copy(tsta[64:65, js], ptr[64:65, :])
                # t = sqrt(64/K + sum sq): one batched Ln, then one Exp per
                # destination row (direct writes; a DMA scatter here would
                # stall behind AllToAll traffic on the DMA engines)
                tlog = smp.tile([65, N], F32, tag="tlog", bufs=2,
                                name=f"tlog{b}")
                nc.scalar.activation(tlog[:], tsta[:], Ln,
                                     bias=bias640[0:65, :])
                trow = smp.tile([65, N], BF16, tag="trow", bufs=2,
                                name=f"trow{b}")
                nc.scalar.activation(trow[:], tlog[:], Exp, scale=0.5)
                # rows scattered by tiny SBUF DMAs on the scalar engine's
                # own HWDGE queue (issue sits right behind the Exp; avoids
                # the shared sync queue where A2A traffic lives)
                nc.scalar.dma_start(qa[64:65, :], trow[0:1, :])
                nc.scalar.dma_start(ka[64:65, :], trow[32:33, :])
                nc.scalar.dma_start(vT[64:65, :], trow[64:65, :])

                # fp8 stage + fold into DoubleRow layout (gpsimd cast, then
                # two partition-remap SBUF DMAs per tensor on the gpsimd
                # HWDGE queue)
                q8s = smp.tile([65, N], FP8, tag="q8s", bufs=2,
                               name=f"q8s{b}")
                k8s = smp.tile([65, N], FP8, tag="k8s", bufs=2,
                               name=f"k8s{b}")
                nc.gpsimd.tensor_copy(q8s[:], qa[:])
                nc.gpsimd.tensor_copy(k8s[:], ka[:])
                qf, kf = qfolds[b % 2], kfolds[b % 2]
                nc.gpsimd.dma_start(qf[0:32, 0, :], q8s[0:32, :])
                nc.gpsimd.dma_start(qf[0:33, 1, :], q8s[32:65, :])
                nc.gpsimd.dma_start(kf[0:32, 0, :], k8s[0:32, :])
                nc.gpsimd.dma_start(kf[0:33, 1, :], k8s[32:65, :])

                # rotate v to token-major [128, 16, 80-pitched] fp8
                va = atp.tile([128, N // 128, 80], FP8, tag="va",
                              bufs=2, name=f"va{b}")
                for j in range(N // 128):
                    pstv = psp.tile([128, 65], BF16, tag="ps",
                                    name=f"pstv{b}_{j}")
                    nc.tensor.transpose(
                        pstv[:], vT[:, j * 128:(j + 1) * 128],
                        identB[0:65, 0:65],
                    )
                    nc.vector.tensor_copy(va[:, j, 0:65], pstv[:])
                qkv[b] = (qf, kf, va)

            # ---- attention + midpoint + per-half AllToAll ----
            def attention(b, mid_cb=None, late_cb=None):
                qf, kf, va = qkv.pop(b)

                # drain: midpoint normalize + send + AllToAll of one half.
                # Called a couple of mi-steps into the NEXT half's loop so
                # its DVE chain (cast/square) hides behind scores matmuls.
                def drain(h2, mts):
                    qoff = h2 * HALF
                    mTb = atp.tile([65, HALF], BF16, tag="mTb", bufs=2,
                                   name=f"mTb{b}_{h2}")
                    nc.vector.tensor_copy(mTb[:], mts[0:65, :])
                    sqb = atp.tile([65, HALF], BF16, tag="sqb", bufs=2,
                                   name=f"sqb{b}_{h2}")
                    nc.vector.tensor_mul(sqb[:], mTb[:], mTb[:])
                    # r = t^2 - |s|^2 via sign-vector matmul, token layout;
                    # reuse head of the (now consumed) mts psum tile
                    for j in range(HALF // 128):
                        nc.tensor.matmul(
                            mts[:, j:j + 1],
                            sqb[:, j * 128:(j + 1) * 128],
                            signv[:],
                            start=True, stop=True,
                        )
                    rl = smp.tile([128, HALF // 128], F32, tag="rl", bufs=2,
                                  name=f"rl{b}_{h2}")
                    nc.scalar.activation(rl[:], mts[:, 0:HALF // 128], Ln,
                                         scale=KCURV)
                    rinv = smp.tile([128, HALF // 128], F32, tag="rinv",
                                    bufs=2, name=f"rinv{b}_{h2}")
                    nc.scalar.activation(rinv[:], rl[:], Exp, scale=-0.5)
                    for g in range(HALF // 512):
                        ms = smp.tile([128, 4, DHS + 1], BF16, tag="ms",
                                      bufs=3, name=f"ms{b}_{h2}_{g}")
                        for jj in range(4):
                            j = g * 4 + jj
                            pstr = psp.tile([128, 65], BF16, tag="ps",
                                            name=f"pstr{b}_{h2}_{j}")
                            nc.tensor.transpose(
                                pstr[:], mTb[:, j * 128:(j + 1) * 128],
                                identB[0:65, 0:65],
                            )
                            nc.vector.tensor_scalar_mul(
                                ms[:, jj, :], pstr[:], rinv[:, j:j + 1]
                            )
                        dst = sends[b][qoff + g * 512:qoff + (g + 1) * 512, :]
                        nc.sync.dma_start(
                            dst.rearrange("(c p) d -> p c d", p=128), ms[:]
                        )
                    # exchange this half while the other half computes
                    nc.gpsimd.collective_compute(
                        "AllToAll",
                        mybir.AluOpType.bypass,
                        replica_groups=[list(range(NCORES))],
                        ins=[sends[b][qoff:qoff + HALF, :].opt()],
                        outs=[recvs[b][h2].opt()],
                    )

                for h2 in range(N // HALF):
                    qoff = h2 * HALF
                    mts = mtp.tile([128, HALF], F32, tag="mt", bufs=1,
                                   name=f"mts{b}_{h2}")
                    # software-pipelined: scores(mi), exp(mi) into pair
                    # tiles; AV consumes a PAIR of key blocks per DoubleRow
                    # matmul, one pair of slack so the PE never waits
                    NMI = N // 128
                    pend = []

                    def flush_av(last):
                        pp, ppt = pend.pop(0)
                        for s in range(HALF // 512):
                            nc.tensor.matmul(
                                mts[0:65, s * 512:(s + 1) * 512],
                                va[:, 2 * pp:2 * pp + 2, 0:65],
                                ppt[:, :, s * 512:(s + 1) * 512],
                                start=(pp == 0), stop=(last and not pend),
                                perf_mode=DR,
                            )

                    ptp = None
                    for mi in range(NMI):
                        ks = slice(mi * 128, (mi + 1) * 128)
                        pss = scp.tile([128, HALF], F32, tag="sc", bufs=2,
                                       name=f"pss{b}_{h2}_{mi}")
                        for s in range(HALF // 512):
                            nc.tensor.matmul(
                                pss[:, s * 512:(s + 1) * 512],
                                kf[:, :, ks],
                                qf[:, :, qoff + s * 512:qoff + (s + 1) * 512],
                                start=True, stop=True, perf_mode=DR,
                            )
                        if mi % 2 == 0:
                            ptp = atp.tile([128, 2, HALF], FP8, tag="pt",
                                           bufs=3, name=f"pt{b}_{h2}_{mi}")
                        nc.scalar.activation(ptp[:, mi % 2, :], pss[:], Exp,
                                             scale=-SCALE / (W8 * W8))
                        if mi % 2 == 1:
                            pend.append((mi // 2, ptp))
                            if len(pend) > 1:
                                flush_av(False)
                        if h2 == 1 and mi == 12 and late_cb is not None:
                            late_cb()
                    while pend:
                        flush_av(True)
                    drain(h2, mts)
                    if h2 == 0 and mid_cb is not None:
                        mid_cb()

            # ---------------- Phase 2 for one batch ----------------
            def ph2_half(b, h, split_y=False):
                rv = d2p.tile([128, NCORES, DHS + 1], BF16, tag="rv",
                              bufs=4, name=f"rv{b}_{h}")
                nc.sync.dma_start(
                    rv[:], recvs[b][h][:].rearrange("j p d -> p j d")
                )
                tsq = smp.tile([128, NCORES], F32, tag="tsq", bufs=2,
                               name=f"tsq{b}_{h}")
                nc.vector.tensor_mul(tsq[:], rv[:, :, 64], rv[:, :, 64])
                tsA = smp.tile([128, 1], F32, tag="tsA", bufs=2,
                               name=f"tsA{b}_{h}")
                nc.vector.reduce_sum(tsA[:], tsq[:],
                                     axis=mybir.AxisListType.X)
                # t' = sqrt(s^2 * sum_h t_h^2 + INVK*(1 + H*s^2))
                lnt = smp.tile([128, 1], F32, tag="lnt", bufs=2,
                               name=f"lnt{b}_{h}")
                nc.scalar.activation(
                    lnt[:], tsA[:], Ln, scale=S_CONST * S_CONST,
                    bias=biasD[:]
                )
                tpA = smp.tile([128, 1], F32, tag="tpA", bufs=2,
                               name=f"tpA{b}_{h}")
                nc.scalar.activation(tpA[:], lnt[:], Exp, scale=0.5)
                fu = d2p.tile([128, DPAD], BF16, tag="fu", bufs=2,
                              name=f"fu{b}_{h}")
                nc.vector.tensor_copy(fu[:, 0:1], tpA[:])
                nc.vector.tensor_scalar_mul(
                    fu[:, 1:513].rearrange("p (j s) -> p j s", j=H),
                    rv[:, :, 0:DHS],
                    S_CONST,
                )
                nc.vector.memset(fu[:, 513:514], 1.0)
                nc.vector.memset(fu[:, 514:DPAD], 0.0)

                ftb = d2p.tile([128, KC, 128], BF16, tag="ftb", bufs=2,
                               name=f"ftb{b}_{h}")
                for ki in range(KC):
                    pstf = psp.tile([128, 128], BF16, tag="ps",
                                    name=f"pstf{b}_{h}_{ki}")
                    nc.tensor.transpose(
                        pstf[:], fu[:, ki * 128:(ki + 1) * 128], identB[:]
                    )
                    nc.vector.tensor_copy(ftb[:, ki, :], pstf[:])

                pso = psp.tile([128, 512], F32, tag="ps",
                               name=f"pso{b}_{h}")
                for ki in range(KC):
                    nc.tensor.matmul(
                        pso[:], ftb[:, ki, :], wob[:, ki, :],
                        start=(ki == 0), stop=(ki == KC - 1),
                    )
                outt = d2p.tile([128, D], F32, tag="outt", bufs=4,
                                name=f"outt{b}_{h}")
                nc.vector.tensor_copy(outt[:, 1:D], pso[:])
                osq = smp.tile([128, 512], BF16, tag="osq", bufs=2,
                               name=f"osq{b}_{h}")
                nc.vector.tensor_mul(osq[:], outt[:, 1:D], outt[:, 1:D])
                osA = smp.tile([128, 1], F32, tag="osA", bufs=2,
                               name=f"osA{b}_{h}")
                nc.vector.reduce_sum(osA[:], osq[:],
                                     axis=mybir.AxisListType.X)
                lno = smp.tile([128, 1], F32, tag="lno", bufs=2,
                               name=f"lno{b}_{h}")
                nc.scalar.activation(lno[:], osA[:], Ln, bias=bias10[:])
                nc.scalar.activation(outt[:, 0:1], lno[:], Exp, scale=0.5)
                r0 = b * TPB + h * 128
                if split_y:
                    # nothing follows in the scalar queue at the very
                    # end, so its HWDGE can take half the store
                    nc.sync.dma_start(y_ap[r0:r0 + 128, 0:256],
                                      outt[:, 0:256])
                    nc.scalar.dma_start(y_ap[r0:r0 + 128, 256:D],
                                        outt[:, 256:D])
                else:
                    nc.sync.dma_start(y_ap[r0:r0 + 128, :], outt[:])

            def phase2(b, split_y=False):
                for h in range(2):
                    ph2_half(b, h, split_y=split_y)

            # ------- schedule: proj lookahead + pipelined A2A/phase2 -------
            xload(0)
            xload(1)
            proj(0)
            for b in range(B):
                if b + 2 < B:
                    xload(b + 2)
                if b + 1 < B:
                    proj(b + 1)
                # phase2(b-1) runs between the halves of attention(b): its
                # exchange finished during half A, and its y stores land
                # mid-batch instead of on the critical tail
                cb = (lambda bb: lambda: phase2(bb))(b - 1) if b >= 1 else None
                attention(b, mid_cb=cb)
            phase2(B - 1, split_y=True)

    nc.compile()
    return nc


def _prep_inputs(x, Wq, bq, Wk, bk, Wv, bv, Wo, bo):
    xT = np.zeros((DPAD, BN), dtype=np.float32)
    xT[:D, :] = np.ascontiguousarray(x.reshape(BN, D).T)
    xT[D, :] = 1.0
    x8 = xT.astype(F8)

    woT = np.zeros((DPAD, D - 1), dtype=np.float32)
    woT[:D + 1, :] = np.concatenate([Wo.T, bo[None, :]], axis=0)
    woTb = woT.astype(BF)

    in_maps = []
    for h in range(NCORES):
        wqk = np.zeros((768, 128), dtype=np.float32)
        wqk[:D + 1, 0:64] = W8 * np.concatenate([Wq[h].T, bq[h][None, :]],
                                                axis=0)
        # negated k: folds the Lorentz score sign into the exp scale
        wqk[:D + 1, 64:128] = -W8 * np.concatenate([Wk[h].T, bk[h][None, :]],
                                                   axis=0)
        wv = np.zeros((768, DHS), dtype=np.float32)
        wv[:D + 1, :] = W8 * np.concatenate([Wv[h].T, bv[h][None, :]],
                                            axis=0)
        # pre-pair the DoubleRow k-tiles: [768, M] -> [128, 3, 2, M]
        wqk8 = np.ascontiguousarray(
            wqk.reshape(3, 2, 128, 128).transpose(2, 0, 1, 3)).astype(F8)
        wv8 = np.ascontiguousarray(
            wv.reshape(3, 2, 128, DHS).transpose(2, 0, 1, 3)).astype(F8)
        in_maps.append({
            "x8": x8,
            "wqk8": wqk8,
            "wv8": wv8,
            "woT": woTb,
        })
    return in_maps


def _run(inputs, trace=False, **kw):
    if "nc" not in _CACHE:
        _CACHE["nc"] = _build()
    nc = _CACHE["nc"]
    in_maps = _prep_inputs(**{k: np.asarray(v) for k, v in inputs.items()})
    res = bass_utils.run_bass_kernel_spmd(
        nc, in_maps, core_ids=list(range(NCORES)), trace=trace, **kw
    )
    y = np.stack([res.results[c]["y"] for c in range(NCORES)], axis=0)
    # y[c, b*256 + h*128 + i, :] holds token b*2048 + h*1024 + c*128 + i
    y = y.reshape(NCORES, B, 2, HTOK, D).transpose(1, 2, 0, 3, 4)
    return np.ascontiguousarray(y.reshape(B, N, D)), res


def kernel(**inputs):
    y, _ = _run(inputs)
    return y


# revision 7
# speedup vs baseline: 1.2048x; 1.2048x over previous
"""Lorentz multi-head attention on 8 Trainium2 NeuronCores (v3, fp8).

Same head-parallel structure as v2 (see git history of this file): core c
computes head c for all batches; per-half-batch AllToAll redistributes
head-blocks to token-blocks; phase 2 (concat_logradius fusion + output
LorentzFC) is woven between the attention halves of the next batch.

v3 moves all phase-1 matmuls to fp8e4m3 DoubleRow (0.5 PE cycles/row):
- host pre-scales Wq/Wk/Wv by 8 (Lorentz midpoint normalization is
  invariant to a common q/k/v scale) so weight values clear the e4m3
  denormal range; the exp scale becomes -SCALE/64 and the t-row
  sqrt bias 1/K becomes 64/K.
- x is fed as fp8 [640, BN] (bias lane 513=1), weights as pre-paired
  k-tiles [128, 3, 2, M]; projections run 3 DoubleRow matmuls per
  512-token chunk (k-tile pair 2 is zero-padded via a once-memset x
  slice).
- q/k land in bf16 [65, N] (squares for the t-rows read them at DVE 4x),
  then a gpsimd cast makes an fp8 copy and two SBUF->SBUF DMAs fold it
  into the DoubleRow layout [33, 2, N] (dims 0-31 + zero pad | dims
  32-63 + t row). Scores = one K=33 DoubleRow matmul per (key-block,
  512 queries).
- exp writes fp8 directly into key-block PAIR tiles [128, 2, 1024]; AV
  consumes a pair per DoubleRow matmul (va pitched [128, 16, 80] since
  dual-fp8 ldweights needs 16B-aligned outer strides).

Midpoint drain, AllToAll, and phase 2 are unchanged from v2 (the
exchanged midpoints are scale-free after r-normalization).
"""

import os
import sys

sys.path.insert(0, "/opt/trn_rl_repo")

import numpy as np
import ml_dtypes

import concourse.bass as bass
import concourse.mybir as mybir
import concourse.tile as tile
from concourse import bacc, bass_utils
from concourse.masks import make_identity

# Problem constants (hardcoded per task contract)
B, N, D = 4, 2048, 513
H, DHS = 8, 64
NCORES = 8
KCURV = 0.1
INVK = 10.0
SCALE = 1.0 / np.sqrt(DHS)  # 0.125
S_CONST = 2.8479428291320801  # exp(0.5*(digamma(256)-digamma(32)))
W8 = 8.0  # host weight pre-scale (folded out by midpoint normalization)
DPAD = 640  # 513 padded to 5*128 (col 513 = constant-1 bias lane)
KC = 5  # contraction chunks of 128
BN = B * N  # 8192 tokens
RPC = BN // NCORES  # 1024 rows per core in phase 2 (256 per batch)
TPB = N // NCORES  # 256 tokens per core per batch
HTOK = TPB // 2  # 128 tokens per core per half-batch A2A
HALF = 1024  # query columns per attention half
F32 = mybir.dt.float32
BF16 = mybir.dt.bfloat16
FP8 = mybir.dt.float8e4
Ln = mybir.ActivationFunctionType.Ln
Exp = mybir.ActivationFunctionType.Exp
DR = mybir.MatmulPerfMode.DoubleRow

_CACHE = {}
BF = ml_dtypes.bfloat16
F8 = ml_dtypes.float8_e4m3


def _patch_act_tables(nc):
    # Exp and Ln both live in the natural_log_exp_and_others set; the
    # table-load pass picks the first set containing each function, which
    # splits them across two sets and reloads tables on every Ln<->Exp
    # switch (~1.3us each). Restrict the map so the combined set wins.
    from concourse.hw_specs import get_activation_tables

    try:
        tabs = get_activation_tables(nc.m.arch)
    except Exception:
        return
    if "natural_log_exp_and_others" not in tabs:
        return
    for name, fns in tabs.items():
        if name != "natural_log_exp_and_others":
            fns.discard(Exp)
            fns.discard(Ln)


def _build():
    nc = bacc.Bacc(
        "TRN2", target_bir_lowering=False, debug=False, num_devices=NCORES
    )
    _patch_act_tables(nc)

    x8_ap = nc.dram_tensor("x8", [DPAD, BN], FP8, kind="ExternalInput").ap()
    wqk_ap = nc.dram_tensor("wqk8", [128, 3, 2, 128], FP8,
                            kind="ExternalInput").ap()
    wv_ap = nc.dram_tensor("wv8", [128, 3, 2, DHS], FP8,
                           kind="ExternalInput").ap()
    woT_ap = nc.dram_tensor("woT", [DPAD, D - 1], BF16,
                            kind="ExternalInput").ap()
    y_ap = nc.dram_tensor("y", [RPC, D], F32, kind="ExternalOutput").ap()

    with tile.TileContext(nc) as tc:
        with (
            tc.tile_pool(name="const", bufs=1) as constp,
            tc.tile_pool(name="w", bufs=1) as wp,
            tc.tile_pool(name="x", bufs=1) as xp,
            tc.tile_pool(name="qk", bufs=1) as qkp,
            tc.tile_pool(name="att", bufs=1) as atp,
            tc.tile_pool(name="sm", bufs=1) as smp,
            tc.tile_pool(name="p2", bufs=1) as d2p,
            tc.tile_pool(name="ps", bufs=2, space="PSUM") as psp,
            tc.tile_pool(name="sc", bufs=2, space="PSUM") as scp,
            tc.tile_pool(name="mt", bufs=1, space="PSUM") as mtp,
            tc.tile_pool(name="dram", bufs=1, space="DRAM") as dramp,
        ):
            identB = constp.tile([128, 128], BF16)
            make_identity(nc, identB[:])
            signv = constp.tile([65, 1], BF16)
            nc.vector.memset(signv[0:64, :], -1.0)
            nc.vector.memset(signv[64:65, :], 1.0)
            # col 0 selects q rows (0-63), col 32 selects k rows (64-127):
            # activation-engine reads must start at partition 0/32/64, so
            # the k t-sum row lands on partition 32
            selqk = constp.tile([128, 33], BF16)
            nc.vector.memset(selqk[:], 0.0)
            nc.vector.memset(selqk[0:64, 0:1], 1.0)
            nc.vector.memset(selqk[64:128, 32:33], 1.0)
            onesv = constp.tile([64, 1], BF16)
            nc.vector.memset(onesv[:], 1.0)
            # t-row bias: 64/K because q/k/v are host-scaled by 8
            bias640 = constp.tile([128, 1], F32)
            nc.vector.memset(bias640[:], INVK * W8 * W8)
            # phase-2 biases use the unscaled 1/K (exchange is scale-free)
            bias10 = constp.tile([128, 1], F32)
            nc.vector.memset(bias10[:], INVK)
            biasD = constp.tile([128, 1], F32)
            nc.vector.memset(biasD[:], INVK * (1.0 + H * S_CONST * S_CONST))

            # Weights: pre-paired DoubleRow k-tiles (host-built fp8)
            wqkb = wp.tile([128, 3, 2, 128], FP8)
            wvb = wp.tile([128, 3, 2, DHS], FP8)
            wob = wp.tile([128, KC, D - 1], BF16)
            nc.sync.dma_start(wqkb[:], wqk_ap)
            nc.sync.dma_start(wvb[:], wv_ap)
            nc.sync.dma_start(wob[:], woT_ap.rearrange("(k p) s -> p k s",
                                                       p=128))

            # x slots: [128, 6, 2048] fp8, ki=5 stays zero (DoubleRow pad)
            x8s = []
            for i in range(3):
                t = xp.tile([128, 6, N], FP8, name=f"x8s{i}")
                nc.vector.memset(t[:, 5, :], 0.0)
                x8s.append(t)

            sends = []
            recvs = []
            for b in range(B):
                sends.append(dramp.tile([N, DHS + 1], BF16, tag=f"send{b}",
                                        name=f"send{b}"))
                recvs.append([
                    dramp.tile([NCORES, HTOK, DHS + 1], BF16,
                               tag=f"recv{b}_{h}", name=f"recv{b}_{h}")
                    for h in range(2)
                ])

            qkv = {}

            # x loads are issued well ahead of each batch so they never
            # queue behind AllToAll traffic on the DMA engines
            def xload(b):
                xt = x8s[b % 3]
                if b != 0:
                    for ki in range(KC):
                        nc.sync.dma_start(
                            xt[:, ki, :],
                            x8_ap[ki * 128:(ki + 1) * 128, b * N:(b + 1) * N],
                        )
                else:
                    # nj-major column-split loads: proj(0) nj=0 needs only
                    # the first 512 columns of each contraction chunk, so
                    # those land first and compute starts early
                    for nj in range(4):
                        js = slice(nj * 512, (nj + 1) * 512)
                        for ki in range(KC):
                            nc.sync.dma_start(
                                xt[:, ki, js],
                                x8_ap[ki * 128:(ki + 1) * 128,
                                      nj * 512:(nj + 1) * 512],
                            )

            # ---- projections (q,k fused; v transposed) + t rows ----
            def proj(b):
                xt = x8s[b % 3]

                qa = qkp.tile([65, N], BF16, tag="qa", bufs=2, name=f"qa{b}")
                ka = qkp.tile([65, N], BF16, tag="ka", bufs=2, name=f"ka{b}")
                vT = qkp.tile([65, N], BF16, tag="vT", bufs=2, name=f"vT{b}")
                # row 0 = q sums, row 32 = k sums, row 64 = v sums
                # (partition-aligned for activation reads; rest is junk)
                tsta = smp.tile([65, N], F32, tag="tsta", bufs=2,
                                name=f"tsta{b}")
                for nj in range(N // 512):
                    js = slice(nj * 512, (nj + 1) * 512)
                    psqk = psp.tile([128, 512], F32, tag="ps",
                                    name=f"pqk{b}_{nj}")
                    for p in range(3):
                        nc.tensor.matmul(
                            psqk[:], wqkb[:, p, :, :],
                            xt[:, 2 * p:2 * p + 2, js],
                            start=(p == 0), stop=(p == 2), perf_mode=DR,
                        )
                    nc.vector.tensor_copy(qa[0:64, js], psqk[0:64, :])
                    nc.vector.tensor_copy(ka[0:64, js], psqk[64:128, :])
                    sqqk = smp.tile([128, 512], BF16, tag="sqqk", bufs=2,
                                    name=f"sqqk{b}_{nj}")
                    nc.vector.tensor_mul(sqqk[0:64, :], qa[0:64, js],
                                         qa[0:64, js])
                    nc.vector.tensor_mul(sqqk[64:128, :], ka[0:64, js],
                                         ka[0:64, js])
                    psv = psp.tile([64, 512], F32, tag="ps",
                                   name=f"pv{b}_{nj}")
                    for p in range(3):
                        nc.tensor.matmul(
                            psv[:], wvb[:, p, :, :],
                            xt[:, 2 * p:2 * p + 2, js],
                            start=(p == 0), stop=(p == 2), perf_mode=DR,
                        )
                    nc.vector.tensor_copy(vT[0:64, js], psv[:])
                    sqv = smp.tile([64, 512], BF16, tag="sqv", bufs=2,
                                   name=f"sqv{b}_{nj}")
                    nc.vector.tensor_mul(sqv[:], vT[0:64, js], vT[0:64, js])
                    ptr = psp.tile([65, 512], F32, tag="ps",
                                   name=f"ptr{b}_{nj}")
                    nc.tensor.matmul(ptr[0:33, :], selqk[:], sqqk[:],
                                     start=True, stop=True)
                    nc.tensor.matmul(ptr[64:65, :], onesv[:], sqv[:],
                                     start=True, stop=True)
                    nc.vector.tensor_copy(tsta[0:33, js], ptr[0:33, :])
                    nc.vector.tensor_copy(tsta[64:65, js], ptr[64:65, :])
                # t = sqrt(64/K + sum sq): one batched Ln, then one Exp per
                # destination row (direct writes; a DMA scatter here would
                # stall behind AllToAll traffic on the DMA engines)
                tlog = smp.tile([65, N], F32, tag="tlog", bufs=2,
                                name=f"tlog{b}")
                nc.scalar.activation(tlog[:], tsta[:], Ln,
                                     bias=bias640[0:65, :])
                trow = smp.tile([65, N], BF16, tag="trow", bufs=2,
                                name=f"trow{b}")
                nc.scalar.activation(trow[:], tlog[:], Exp, scale=0.5)
                # rows scattered by tiny SBUF DMAs on the scalar engine's
                # own HWDGE queue (issue sits right behind the Exp; avoids
                # the shared sync queue where A2A traffic lives)
                nc.scalar.dma_start(qa[64:65, :], trow[0:1, :])
                nc.scalar.dma_start(ka[64:65, :], trow[32:33, :])
                nc.scalar.dma_start(vT[64:65, :], trow[64:65, :])

                # rotate v to token-major [128, 16, 80-pitched] fp8
                va = atp.tile([128, N // 128, 80], FP8, tag="va",
                              bufs=2, name=f"va{b}")
                for j in range(N // 128):
                    pstv = psp.tile([128, 65], BF16, tag="ps",
                                    name=f"pstv{b}_{j}")
                    nc.tensor.transpose(
                        pstv[:], vT[:, j * 128:(j + 1) * 128],
                        identB[0:65, 0:65],
                    )
                    nc.vector.tensor_copy(va[:, j, 0:65], pstv[:])
                qkv[b] = (qa, ka, va)

            # ---- attention + midpoint + per-half AllToAll ----
            def attention(b, mid_cb=None, late_cb=None):
                qa, ka, va = qkv.pop(b)

                # drain: midpoint normalize + send + AllToAll of one half.
                # Called a couple of mi-steps into the NEXT half's loop so
                # its DVE chain (cast/square) hides behind scores matmuls.
                def drain(h2, mts):
                    qoff = h2 * HALF
                    mTb = atp.tile([65, HALF], BF16, tag="mTb", bufs=2,
                                   name=f"mTb{b}_{h2}")
                    nc.vector.tensor_copy(mTb[:], mts[0:65, :])
                    sqb = atp.tile([65, HALF], BF16, tag="sqb", bufs=2,
                                   name=f"sqb{b}_{h2}")
                    nc.vector.tensor_mul(sqb[:], mTb[:], mTb[:])
                    # r = t^2 - |s|^2 via sign-vector matmul, token layout;
                    # reuse head of the (now consumed) mts psum tile
                    for j in range(HALF // 128):
                        nc.tensor.matmul(
                            mts[:, j:j + 1],
                            sqb[:, j * 128:(j + 1) * 128],
                            signv[:],
                            start=True, stop=True,
                        )
                    rl = smp.tile([128, HALF // 128], F32, tag="rl", bufs=2,
                                  name=f"rl{b}_{h2}")
                    nc.scalar.activation(rl[:], mts[:, 0:HALF // 128], Ln,
                                         scale=KCURV)
                    rinv = smp.tile([128, HALF // 128], F32, tag="rinv",
                                    bufs=2, name=f"rinv{b}_{h2}")
                    nc.scalar.activation(rinv[:], rl[:], Exp, scale=-0.5)
                    for g in range(HALF // 512):
                        ms = smp.tile([128, 4, DHS + 1], BF16, tag="ms",
                                      bufs=3, name=f"ms{b}_{h2}_{g}")
                        for jj in range(4):
                            j = g * 4 + jj
                            pstr = psp.tile([128, 65], BF16, tag="ps",
                                            name=f"pstr{b}_{h2}_{j}")
                            nc.tensor.transpose(
                                pstr[:], mTb[:, j * 128:(j + 1) * 128],
                                identB[0:65, 0:65],
                            )
                            nc.vector.tensor_scalar_mul(
                                ms[:, jj, :], pstr[:], rinv[:, j:j + 1]
                            )
                        dst = sends[b][qoff + g * 512:qoff + (g + 1) * 512, :]
                        nc.sync.dma_start(
                            dst.rearrange("(c p) d -> p c d", p=128), ms[:]
                        )
                    # exchange this half while the other half computes
                    nc.gpsimd.collective_compute(
                        "AllToAll",
                        mybir.AluOpType.bypass,
                        replica_groups=[list(range(NCORES))],
                        ins=[sends[b][qoff:qoff + HALF, :].opt()],
                        outs=[recvs[b][h2].opt()],
                    )

                for h2 in range(N // HALF):
                    qoff = h2 * HALF
                    mts = mtp.tile([128, HALF], F32, tag="mt", bufs=1,
                                   name=f"mts{b}_{h2}")
                    # software-pipelined: scores(mi), exp(mi) into pair
                    # tiles; AV consumes a PAIR of key blocks per DoubleRow
                    # matmul, one pair of slack so the PE never waits
                    NMI = N // 128
                    pend = []

                    def flush_av(last):
                        pp, ppt = pend.pop(0)
                        for s in range(HALF // 512):
                            nc.tensor.matmul(
                                mts[0:65, s * 512:(s + 1) * 512],
                                va[:, 2 * pp:2 * pp + 2, 0:65],
                                ppt[:, :, s * 512:(s + 1) * 512],
                                start=(pp == 0), stop=(last and not pend),
                                perf_mode=DR,
                            )

                    ptp = None
                    for mi in range(NMI):
                        ks = slice(mi * 128, (mi + 1) * 128)
                        pss = scp.tile([128, HALF], F32, tag="sc", bufs=2,
                                       name=f"pss{b}_{h2}_{mi}")
                        for s in range(HALF // 512):
                            nc.tensor.matmul(
                                pss[:, s * 512:(s + 1) * 512],
                                ka[:, ks],
                                qa[:, qoff + s * 512:qoff + (s + 1) * 512],
                                start=True, stop=True,
                            )
                        if mi % 2 == 0:
                            ptp = atp.tile([128, 2, HALF], FP8, tag="pt",
                                           bufs=3, name=f"pt{b}_{h2}_{mi}")
                        nc.scalar.activation(ptp[:, mi % 2, :], pss[:], Exp,
                                             scale=-SCALE / (W8 * W8))
                        if mi % 2 == 1:
                            pend.append((mi // 2, ptp))
                            if len(pend) > 1:
                                flush_av(False)
                        if h2 == 1 and mi == 12 and late_cb is not None:
                            late_cb()
                    while pend:
                        flush_av(True)
                    drain(h2, mts)
                    if h2 == 0 and mid_cb is not None:
                        mid_cb()

            # ---------------- Phase 2 for one batch ----------------
            def ph2_half(b, h, split_y=False):
                rv = d2p.tile([128, NCORES, DHS + 1], BF16, tag="rv",
                              bufs=4, name=f"rv{b}_{h}")
                nc.sync.dma_start(
                    rv[:], recvs[b][h][:].rearrange("j p d -> p j d")
                )
                tsq = smp.tile([128, NCORES], F32, tag="tsq", bufs=2,
                               name=f"tsq{b}_{h}")
                nc.vector.tensor_mul(tsq[:], rv[:, :, 64], rv[:, :, 64])
                tsA = smp.tile([128, 1], F32, tag="tsA", bufs=2,
                               name=f"tsA{b}_{h}")
                nc.vector.reduce_sum(tsA[:], tsq[:],
                                     axis=mybir.AxisListType.X)
                # t' = sqrt(s^2 * sum_h t_h^2 + INVK*(1 + H*s^2))
                lnt = smp.tile([128, 1], F32, tag="lnt", bufs=2,
                               name=f"lnt{b}_{h}")
                nc.scalar.activation(
                    lnt[:], tsA[:], Ln, scale=S_CONST * S_CONST,
                    bias=biasD[:]
                )
                tpA = smp.tile([128, 1], F32, tag="tpA", bufs=2,
                               name=f"tpA{b}_{h}")
                nc.scalar.activation(tpA[:], lnt[:], Exp, scale=0.5)
                fu = d2p.tile([128, DPAD], BF16, tag="fu", bufs=2,
                              name=f"fu{b}_{h}")
                nc.vector.tensor_copy(fu[:, 0:1], tpA[:])
                nc.vector.tensor_scalar_mul(
                    fu[:, 1:513].rearrange("p (j s) -> p j s", j=H),
                    rv[:, :, 0:DHS],
                    S_CONST,
                )
                nc.vector.memset(fu[:, 513:514], 1.0)
                nc.vector.memset(fu[:, 514:DPAD], 0.0)

                ftb = d2p.tile([128, KC, 128], BF16, tag="ftb", bufs=2,
                               name=f"ftb{b}_{h}")
                for ki in range(KC):
                    pstf = psp.tile([128, 128], BF16, tag="ps",
                                    name=f"pstf{b}_{h}_{ki}")
                    nc.tensor.transpose(
                        pstf[:], fu[:, ki * 128:(ki + 1) * 128], identB[:]
                    )
                    nc.vector.tensor_copy(ftb[:, ki, :], pstf[:])

                pso = psp.tile([128, 512], F32, tag="ps",
                               name=f"pso{b}_{h}")
                for ki in range(KC):
                    nc.tensor.matmul(
                        pso[:], ftb[:, ki, :], wob[:, ki, :],
                        start=(ki == 0), stop=(ki == KC - 1),
                    )
                outt = d2p.tile([128, D], F32, tag="outt", bufs=4,
                                name=f"outt{b}_{h}")
                nc.vector.tensor_copy(outt[:, 1:D], pso[:])
                osq = smp.tile([128, 512], BF16, tag="osq", bufs=2,
                               name=f"osq{b}_{h}")
                nc.vector.tensor_mul(osq[:], outt[:, 1:D], outt[:, 1:D])
                osA = smp.tile([128, 1], F32, tag="osA", bufs=2,
                               name=f"osA{b}_{h}")
                nc.vector.reduce_sum(osA[:], osq[:],
                                     axis=mybir.AxisListType.X)
                lno = smp.tile([128, 1], F32, tag="lno", bufs=2,
                               name=f"lno{b}_{h}")
                nc.scalar.activation(lno[:], osA[:], Ln, bias=bias10[:])
                nc.scalar.activation(outt[:, 0:1], lno[:], Exp, scale=0.5)
                r0 = b * TPB + h * 128
                if split_y:
                    # nothing follows in the scalar queue at the very
                    # end, so its HWDGE can take half the store
                    nc.sync.dma_start(y_ap[r0:r0 + 128, 0:256],
                                      outt[:, 0:256])
                    nc.scalar.dma_start(y_ap[r0:r0 + 128, 256:D],
                                        outt[:, 256:D])
                else:
                    nc.sync.dma_start(y_ap[r0:r0 + 128, :], outt[:])

            def phase2(b, split_y=False):
                for h in range(2):
                    ph2_half(b, h, split_y=split_y)

            # ------- schedule: proj lookahead + pipelined A2A/phase2 -------
            xload(0)
            xload(1)
            proj(0)
            for b in range(B):
                if b + 2 < B:
                    xload(b + 2)
                if b + 1 < B:
                    proj(b + 1)
                # phase2(b-1) runs between the halves of attention(b): its
                # exchange finished during half A, and its y stores land
                # mid-batch instead of on the critical tail
                cb = (lambda bb: lambda: phase2(bb))(b - 1) if b >= 1 else None
                attention(b, mid_cb=cb)
            phase2(B - 1, split_y=True)

    nc.compile()
    return nc


def _prep_inputs(x, Wq, bq, Wk, bk, Wv, bv, Wo, bo):
    xT = np.zeros((DPAD, BN), dtype=np.float32)
    xT[:D, :] = np.ascontiguousarray(x.reshape(BN, D).T)
    xT[D, :] = 1.0
    x8 = xT.astype(F8)

    woT = np.zeros((DPAD, D - 1), dtype=np.float32)
    woT[:D + 1, :] = np.concatenate([Wo.T, bo[None, :]], axis=0)
    woTb = woT.astype(BF)

    in_maps = []
    for h in range(NCORES):
        wqk = np.zeros((768, 128), dtype=np.float32)
        wqk[:D + 1, 0:64] = W8 * np.concatenate([Wq[h].T, bq[h][None, :]],
                                                axis=0)
        # negated k: folds the Lorentz score sign into the exp scale
        wqk[:D + 1, 64:128] = -W8 * np.concatenate([Wk[h].T, bk[h][None, :]],
                                                   axis=0)
        wv = np.zeros((768, DHS), dtype=np.float32)
        wv[:D + 1, :] = W8 * np.concatenate([Wv[h].T, bv[h][None, :]],
                                            axis=0)
        # pre-pair the DoubleRow k-tiles: [768, M] -> [128, 3, 2, M]
        wqk8 = np.ascontiguousarray(
            wqk.reshape(3, 2, 128, 128).transpose(2, 0, 1, 3)).astype(F8)
        wv8 = np.ascontiguousarray(
            wv.reshape(3, 2, 128, DHS).transpose(2, 0, 1, 3)).astype(F8)
        in_maps.append({
            "x8": x8,
            "wqk8": wqk8,
            "wv8": wv8,
            "woT": woTb,
        })
    return in_maps


def _run(inputs, trace=False, **kw):
    if "nc" not in _CACHE:
        _CACHE["nc"] = _build()
    nc = _CACHE["nc"]
    in_maps = _prep_inputs(**{k: np.asarray(v) for k, v in inputs.items()})
    res = bass_utils.run_bass_kernel_spmd(
        nc, in_maps, core_ids=list(range(NCORES)), trace=trace, **kw
    )
    y = np.stack([res.results[c]["y"] for c in range(NCORES)], axis=0)
    # y[c, b*256 + h*128 + i, :] holds token b*2048 + h*1024 + c*128 + i
    y = y.reshape(NCORES, B, 2, HTOK, D).transpose(1, 2, 0, 3, 4)
    return np.ascontiguousarray(y.reshape(B, N, D)), res


def kernel(**inputs):
    y, _ = _run(inputs)
    return y


# revision 8
# speedup vs baseline: 1.2398x; 1.0290x over previous
"""Lorentz multi-head attention on 8 Trainium2 NeuronCores (v3, fp8).

Same head-parallel structure as v2 (see git history of this file): core c
computes head c for all batches; per-half-batch AllToAll redistributes
head-blocks to token-blocks; phase 2 (concat_logradius fusion + output
LorentzFC) is woven between the attention halves of the next batch.

v3 moves all phase-1 matmuls to fp8e4m3 DoubleRow (0.5 PE cycles/row):
- host pre-scales Wq/Wk/Wv by 8 (Lorentz midpoint normalization is
  invariant to a common q/k/v scale) so weight values clear the e4m3
  denormal range; the exp scale becomes -SCALE/64 and the t-row
  sqrt bias 1/K becomes 64/K.
- x is fed as fp8 [640, BN] (bias lane 513=1), weights as pre-paired
  k-tiles [128, 3, 2, M]; projections run 3 DoubleRow matmuls per
  512-token chunk (k-tile pair 2 is zero-padded via a once-memset x
  slice).
- q/k land in bf16 [65, N] (squares for the t-rows read them at DVE 4x),
  then a gpsimd cast makes an fp8 copy and two SBUF->SBUF DMAs fold it
  into the DoubleRow layout [33, 2, N] (dims 0-31 + zero pad | dims
  32-63 + t row). Scores = one K=33 DoubleRow matmul per (key-block,
  512 queries).
- exp writes fp8 directly into key-block PAIR tiles [128, 2, 1024]; AV
  consumes a pair per DoubleRow matmul (va pitched [128, 16, 80] since
  dual-fp8 ldweights needs 16B-aligned outer strides).

Midpoint drain, AllToAll, and phase 2 are unchanged from v2 (the
exchanged midpoints are scale-free after r-normalization).
"""

import os
import sys

sys.path.insert(0, "/opt/trn_rl_repo")

import numpy as np
import ml_dtypes

import concourse.bass as bass
import concourse.mybir as mybir
import concourse.tile as tile
from concourse import bacc, bass_utils
from concourse.masks import make_identity

# Problem constants (hardcoded per task contract)
B, N, D = 4, 2048, 513
H, DHS = 8, 64
NCORES = 8
KCURV = 0.1
INVK = 10.0
SCALE = 1.0 / np.sqrt(DHS)  # 0.125
S_CONST = 2.8479428291320801  # exp(0.5*(digamma(256)-digamma(32)))
W8 = 8.0  # host weight pre-scale (folded out by midpoint normalization)
DPAD = 640  # 513 padded to 5*128 (col 513 = constant-1 bias lane)
KC = 5  # contraction chunks of 128
BN = B * N  # 8192 tokens
RPC = BN // NCORES  # 1024 rows per core in phase 2 (256 per batch)
TPB = N // NCORES  # 256 tokens per core per batch
HTOK = TPB // 2  # 128 tokens per core per half-batch A2A
HALF = 1024  # query columns per attention half
F32 = mybir.dt.float32
BF16 = mybir.dt.bfloat16
FP8 = mybir.dt.float8e4
Ln = mybir.ActivationFunctionType.Ln
Exp = mybir.ActivationFunctionType.Exp
DR = mybir.MatmulPerfMode.DoubleRow

_CACHE = {}
BF = ml_dtypes.bfloat16
F8 = ml_dtypes.float8_e4m3


def _patch_act_tables(nc):
    # Exp and Ln both live in the natural_log_exp_and_others set; the
    # table-load pass picks the first set containing each function, which
    # splits them across two sets and reloads tables on every Ln<->Exp
    # switch (~1.3us each). Restrict the map so the combined set wins.
    from concourse.hw_specs import get_activation_tables

    try:
        tabs = get_activation_tables(nc.m.arch)
    except Exception:
        return
    if "natural_log_exp_and_others" not in tabs:
        return
    for name, fns in tabs.items():
        if name != "natural_log_exp_and_others":
            fns.discard(Exp)
            fns.discard(Ln)


def _build():
    nc = bacc.Bacc(
        "TRN2", target_bir_lowering=False, debug=False, num_devices=NCORES
    )
    _patch_act_tables(nc)

    x8_ap = nc.dram_tensor("x8", [DPAD, BN], FP8, kind="ExternalInput").ap()
    wqk_ap = nc.dram_tensor("wqk8", [128, 3, 2, 128], FP8,
                            kind="ExternalInput").ap()
    wv_ap = nc.dram_tensor("wv8", [128, 3, 2, DHS], FP8,
                           kind="ExternalInput").ap()
    woT_ap = nc.dram_tensor("woT", [DPAD, D - 1], BF16,
                            kind="ExternalInput").ap()
    y_ap = nc.dram_tensor("y", [RPC, D], F32, kind="ExternalOutput").ap()

    with tile.TileContext(nc) as tc:
        with (
            tc.tile_pool(name="const", bufs=1) as constp,
            tc.tile_pool(name="w", bufs=1) as wp,
            tc.tile_pool(name="x", bufs=1) as xp,
            tc.tile_pool(name="qk", bufs=1) as qkp,
            tc.tile_pool(name="att", bufs=1) as atp,
            tc.tile_pool(name="sm", bufs=1) as smp,
            tc.tile_pool(name="p2", bufs=1) as d2p,
            tc.tile_pool(name="ps", bufs=2, space="PSUM") as psp,
            tc.tile_pool(name="sc", bufs=2, space="PSUM") as scp,
            tc.tile_pool(name="mt", bufs=1, space="PSUM") as mtp,
            tc.tile_pool(name="dram", bufs=1, space="DRAM") as dramp,
        ):
            identB = constp.tile([128, 128], BF16)
            make_identity(nc, identB[:])
            signv = constp.tile([65, 1], BF16)
            nc.vector.memset(signv[0:64, :], -1.0)
            nc.vector.memset(signv[64:65, :], 1.0)
            # col 0 selects q rows (0-63), col 32 selects k rows (64-127):
            # activation-engine reads must start at partition 0/32/64, so
            # the k t-sum row lands on partition 32
            selqk = constp.tile([128, 33], BF16)
            nc.vector.memset(selqk[:], 0.0)
            nc.vector.memset(selqk[0:64, 0:1], 1.0)
            nc.vector.memset(selqk[64:128, 32:33], 1.0)
            onesv = constp.tile([64, 1], BF16)
            nc.vector.memset(onesv[:], 1.0)
            # t-row bias: 64/K because q/k/v are host-scaled by 8
            bias640 = constp.tile([128, 1], F32)
            nc.vector.memset(bias640[:], INVK * W8 * W8)
            # phase-2 biases use the unscaled 1/K (exchange is scale-free)
            bias10 = constp.tile([128, 1], F32)
            nc.vector.memset(bias10[:], INVK)
            biasD = constp.tile([128, 1], F32)
            nc.vector.memset(biasD[:], INVK * (1.0 + H * S_CONST * S_CONST))

            # Weights: pre-paired DoubleRow k-tiles (host-built fp8)
            wqkb = wp.tile([128, 3, 2, 128], FP8)
            wvb = wp.tile([128, 3, 2, DHS], FP8)
            wob = wp.tile([128, KC, D - 1], BF16)
            nc.sync.dma_start(wqkb[:], wqk_ap)
            nc.sync.dma_start(wvb[:], wv_ap)
            nc.sync.dma_start(wob[:], woT_ap.rearrange("(k p) s -> p k s",
                                                       p=128))

            # x slots: [128, 6, 2048] fp8, ki=5 stays zero (DoubleRow pad)
            x8s = []
            for i in range(3):
                t = xp.tile([128, 6, N], FP8, name=f"x8s{i}")
                nc.vector.memset(t[:, 5, :], 0.0)
                x8s.append(t)

            sends = []
            recvs = []
            for b in range(B):
                sends.append(dramp.tile([N, DHS + 1], BF16, tag=f"send{b}",
                                        name=f"send{b}"))
                recvs.append([
                    dramp.tile([NCORES, HTOK, DHS + 1], BF16,
                               tag=f"recv{b}_{h}", name=f"recv{b}_{h}")
                    for h in range(2)
                ])

            qkv = {}

            # x loads are issued well ahead of each batch so they never
            # queue behind AllToAll traffic on the DMA engines
            def xload(b):
                xt = x8s[b % 3]
                if b != 0:
                    for ki in range(KC):
                        nc.sync.dma_start(
                            xt[:, ki, :],
                            x8_ap[ki * 128:(ki + 1) * 128, b * N:(b + 1) * N],
                        )
                else:
                    # nj-major column-split loads: proj(0) nj=0 needs only
                    # the first 512 columns of each contraction chunk, so
                    # those land first and compute starts early
                    for nj in range(4):
                        js = slice(nj * 512, (nj + 1) * 512)
                        for ki in range(KC):
                            nc.sync.dma_start(
                                xt[:, ki, js],
                                x8_ap[ki * 128:(ki + 1) * 128,
                                      nj * 512:(nj + 1) * 512],
                            )

            # ---- projections (q,k fused; v transposed) + t rows ----
            def proj(b):
                xt = x8s[b % 3]

                qa = qkp.tile([65, N], BF16, tag="qa", bufs=2, name=f"qa{b}")
                ka = qkp.tile([65, N], BF16, tag="ka", bufs=2, name=f"ka{b}")
                vT = qkp.tile([65, N], BF16, tag="vT", bufs=2, name=f"vT{b}")
                # row 0 = q sums, row 32 = k sums, row 64 = v sums
                # (partition-aligned for activation reads; rest is junk)
                tsta = smp.tile([65, N], F32, tag="tsta", bufs=2,
                                name=f"tsta{b}")
                for nj in range(N // 512):
                    js = slice(nj * 512, (nj + 1) * 512)
                    psqk = psp.tile([128, 512], F32, tag="ps",
                                    name=f"pqk{b}_{nj}")
                    for p in range(3):
                        nc.tensor.matmul(
                            psqk[:], wqkb[:, p, :, :],
                            xt[:, 2 * p:2 * p + 2, js],
                            start=(p == 0), stop=(p == 2), perf_mode=DR,
                        )
                    nc.vector.tensor_copy(qa[0:64, js], psqk[0:64, :])
                    nc.vector.tensor_copy(ka[0:64, js], psqk[64:128, :])
                    sqqk = smp.tile([128, 512], BF16, tag="sqqk", bufs=2,
                                    name=f"sqqk{b}_{nj}")
                    nc.vector.tensor_mul(sqqk[0:64, :], qa[0:64, js],
                                         qa[0:64, js])
                    nc.vector.tensor_mul(sqqk[64:128, :], ka[0:64, js],
                                         ka[0:64, js])
                    psv = psp.tile([64, 512], F32, tag="ps",
                                   name=f"pv{b}_{nj}")
                    for p in range(3):
                        nc.tensor.matmul(
                            psv[:], wvb[:, p, :, :],
                            xt[:, 2 * p:2 * p + 2, js],
                            start=(p == 0), stop=(p == 2), perf_mode=DR,
                        )
                    nc.vector.tensor_copy(vT[0:64, js], psv[:])
                    sqv = smp.tile([64, 512], BF16, tag="sqv", bufs=2,
                                   name=f"sqv{b}_{nj}")
                    nc.vector.tensor_mul(sqv[:], vT[0:64, js], vT[0:64, js])
                    ptr = psp.tile([65, 512], F32, tag="ps",
                                   name=f"ptr{b}_{nj}")
                    nc.tensor.matmul(ptr[0:33, :], selqk[:], sqqk[:],
                                     start=True, stop=True)
                    nc.tensor.matmul(ptr[64:65, :], onesv[:], sqv[:],
                                     start=True, stop=True)
                    nc.vector.tensor_copy(tsta[0:33, js], ptr[0:33, :])
                    nc.vector.tensor_copy(tsta[64:65, js], ptr[64:65, :])
                # t = sqrt(64/K + sum sq): one batched Ln, then one Exp per
                # destination row (direct writes; a DMA scatter here would
                # stall behind AllToAll traffic on the DMA engines)
                tlog = smp.tile([65, N], F32, tag="tlog", bufs=2,
                                name=f"tlog{b}")
                nc.scalar.activation(tlog[:], tsta[:], Ln,
                                     bias=bias640[0:65, :])
                trow = smp.tile([65, N], BF16, tag="trow", bufs=2,
                                name=f"trow{b}")
                nc.scalar.activation(trow[:], tlog[:], Exp, scale=0.5)
                # rows scattered by tiny SBUF DMAs on the scalar engine's
                # own HWDGE queue (issue sits right behind the Exp; avoids
                # the shared sync queue where A2A traffic lives)
                nc.scalar.dma_start(qa[64:65, :], trow[0:1, :])
                nc.scalar.dma_start(ka[64:65, :], trow[32:33, :])
                nc.scalar.dma_start(vT[64:65, :], trow[64:65, :])

                # rotate v to token-major [128, 16, 80-pitched] fp8
                va = atp.tile([128, N // 128, 80], FP8, tag="va",
                              bufs=2, name=f"va{b}")
                for j in range(N // 128):
                    pstv = psp.tile([128, 65], BF16, tag="ps",
                                    name=f"pstv{b}_{j}")
                    nc.tensor.transpose(
                        pstv[:], vT[:, j * 128:(j + 1) * 128],
                        identB[0:65, 0:65],
                    )
                    nc.vector.tensor_copy(va[:, j, 0:65], pstv[:])
                qkv[b] = (qa, ka, va)

            # ---- attention + midpoint + per-half AllToAll ----
            def attention(b, mid_cb=None, late_cb=None):
                qa, ka, va = qkv.pop(b)

                # drain: midpoint normalize + send + AllToAll of one half.
                # Called a couple of mi-steps into the NEXT half's loop so
                # its DVE chain (cast/square) hides behind scores matmuls.
                def drain(h2, mts):
                    qoff = h2 * HALF
                    mTb = atp.tile([65, HALF], BF16, tag="mTb", bufs=2,
                                   name=f"mTb{b}_{h2}")
                    nc.vector.tensor_copy(mTb[:], mts[0:65, :])
                    sqb = atp.tile([65, HALF], BF16, tag="sqb", bufs=2,
                                   name=f"sqb{b}_{h2}")
                    nc.vector.tensor_mul(sqb[:], mTb[:], mTb[:])
                    # r = t^2 - |s|^2 via sign-vector matmul, token layout;
                    # reuse head of the (now consumed) mts psum tile
                    for j in range(HALF // 128):
                        nc.tensor.matmul(
                            mts[:, j:j + 1],
                            sqb[:, j * 128:(j + 1) * 128],
                            signv[:],
                            start=True, stop=True,
                        )
                    rl = smp.tile([128, HALF // 128], F32, tag="rl", bufs=2,
                                  name=f"rl{b}_{h2}")
                    nc.scalar.activation(rl[:], mts[:, 0:HALF // 128], Ln,
                                         scale=KCURV)
                    rinv = smp.tile([128, HALF // 128], F32, tag="rinv",
                                    bufs=2, name=f"rinv{b}_{h2}")
                    nc.scalar.activation(rinv[:], rl[:], Exp, scale=-0.5)
                    for g in range(HALF // 512):
                        ms = smp.tile([128, 4, DHS + 1], BF16, tag="ms",
                                      bufs=3, name=f"ms{b}_{h2}_{g}")
                        for jj in range(4):
                            j = g * 4 + jj
                            pstr = psp.tile([128, 65], BF16, tag="ps",
                                            name=f"pstr{b}_{h2}_{j}")
                            nc.tensor.transpose(
                                pstr[:], mTb[:, j * 128:(j + 1) * 128],
                                identB[0:65, 0:65],
                            )
                            nc.vector.tensor_scalar_mul(
                                ms[:, jj, :], pstr[:], rinv[:, j:j + 1]
                            )
                        dst = sends[b][qoff + g * 512:qoff + (g + 1) * 512, :]
                        nc.sync.dma_start(
                            dst.rearrange("(c p) d -> p c d", p=128), ms[:]
                        )
                    # exchange this half while the other half computes
                    nc.gpsimd.collective_compute(
                        "AllToAll",
                        mybir.AluOpType.bypass,
                        replica_groups=[list(range(NCORES))],
                        ins=[sends[b][qoff:qoff + HALF, :].opt()],
                        outs=[recvs[b][h2].opt()],
                    )

                for h2 in range(N // HALF):
                    qoff = h2 * HALF
                    mts = mtp.tile([128, HALF], F32, tag="mt", bufs=1,
                                   name=f"mts{b}_{h2}")
                    # software-pipelined: scores(mi), exp(mi) into pair
                    # tiles; AV consumes a PAIR of key blocks per DoubleRow
                    # matmul, one pair of slack so the PE never waits
                    NMI = N // 128
                    pend = []

                    def flush_av(last):
                        pp, ppt = pend.pop(0)
                        for s in range(HALF // 512):
                            nc.tensor.matmul(
                                mts[0:65, s * 512:(s + 1) * 512],
                                va[:, 2 * pp:2 * pp + 2, 0:65],
                                ppt[:, :, s * 512:(s + 1) * 512],
                                start=(pp == 0), stop=(last and not pend),
                                perf_mode=DR,
                            )

                    ptp = None
                    for mi in range(NMI):
                        ks = slice(mi * 128, (mi + 1) * 128)
                        pss = scp.tile([128, HALF], F32, tag="sc", bufs=2,
                                       name=f"pss{b}_{h2}_{mi}")
                        for s in range(HALF // 512):
                            nc.tensor.matmul(
                                pss[:, s * 512:(s + 1) * 512],
                                ka[:, ks],
                                qa[:, qoff + s * 512:qoff + (s + 1) * 512],
                                start=True, stop=True,
                            )
                        if mi % 2 == 0:
                            ptp = atp.tile([128, 2, HALF], FP8, tag="pt",
                                           bufs=3, name=f"pt{b}_{h2}_{mi}")
                        nc.scalar.activation(ptp[:, mi % 2, :], pss[:], Exp,
                                             scale=-SCALE / (W8 * W8))
                        if mi % 2 == 1:
                            pend.append((mi // 2, ptp))
                            if len(pend) > 1:
                                flush_av(False)
                        if h2 == 1 and mi == 12 and late_cb is not None:
                            late_cb()
                    while pend:
                        flush_av(True)
                    drain(h2, mts)
                    if h2 == 0 and mid_cb is not None:
                        mid_cb()

            # ---------------- Phase 2 for one batch ----------------
            def ph2_half(b, h, split_y=False):
                rv = d2p.tile([128, NCORES, DHS + 1], BF16, tag="rv",
                              bufs=4, name=f"rv{b}_{h}")
                nc.sync.dma_start(
                    rv[:], recvs[b][h][:].rearrange("j p d -> p j d")
                )
                tsq = smp.tile([128, NCORES], F32, tag="tsq", bufs=2,
                               name=f"tsq{b}_{h}")
                nc.vector.tensor_mul(tsq[:], rv[:, :, 64], rv[:, :, 64])
                tsA = smp.tile([128, 1], F32, tag="tsA", bufs=2,
                               name=f"tsA{b}_{h}")
                nc.vector.reduce_sum(tsA[:], tsq[:],
                                     axis=mybir.AxisListType.X)
                # t' = sqrt(s^2 * sum_h t_h^2 + INVK*(1 + H*s^2))
                lnt = smp.tile([128, 1], F32, tag="lnt", bufs=2,
                               name=f"lnt{b}_{h}")
                nc.scalar.activation(
                    lnt[:], tsA[:], Ln, scale=S_CONST * S_CONST,
                    bias=biasD[:]
                )
                tpA = smp.tile([128, 1], F32, tag="tpA", bufs=2,
                               name=f"tpA{b}_{h}")
                nc.scalar.activation(tpA[:], lnt[:], Exp, scale=0.5)
                fu = d2p.tile([128, DPAD], BF16, tag="fu", bufs=2,
                              name=f"fu{b}_{h}")
                nc.vector.tensor_copy(fu[:, 0:1], tpA[:])
                nc.vector.tensor_scalar_mul(
                    fu[:, 1:513].rearrange("p (j s) -> p j s", j=H),
                    rv[:, :, 0:DHS],
                    S_CONST,
                )
                nc.vector.memset(fu[:, 513:514], 1.0)
                nc.vector.memset(fu[:, 514:DPAD], 0.0)

                ftb = d2p.tile([128, KC, 128], BF16, tag="ftb", bufs=2,
                               name=f"ftb{b}_{h}")
                for ki in range(KC):
                    pstf = psp.tile([128, 128], BF16, tag="ps",
                                    name=f"pstf{b}_{h}_{ki}")
                    nc.tensor.transpose(
                        pstf[:], fu[:, ki * 128:(ki + 1) * 128], identB[:]
                    )
                    nc.vector.tensor_copy(ftb[:, ki, :], pstf[:])

                pso = psp.tile([128, 512], F32, tag="ps",
                               name=f"pso{b}_{h}")
                for ki in range(KC):
                    nc.tensor.matmul(
                        pso[:], ftb[:, ki, :], wob[:, ki, :],
                        start=(ki == 0), stop=(ki == KC - 1),
                    )
                outt = d2p.tile([128, D], F32, tag="outt", bufs=4,
                                name=f"outt{b}_{h}")
                nc.vector.tensor_copy(outt[:, 1:D], pso[:])
                osq = smp.tile([128, 512], BF16, tag="osq", bufs=2,
                               name=f"osq{b}_{h}")
                nc.vector.tensor_mul(osq[:], outt[:, 1:D], outt[:, 1:D])
                osA = smp.tile([128, 1], F32, tag="osA", bufs=2,
                               name=f"osA{b}_{h}")
                nc.vector.reduce_sum(osA[:], osq[:],
                                     axis=mybir.AxisListType.X)
                lno = smp.tile([128, 1], F32, tag="lno", bufs=2,
                               name=f"lno{b}_{h}")
                nc.scalar.activation(lno[:], osA[:], Ln, bias=bias10[:])
                nc.scalar.activation(outt[:, 0:1], lno[:], Exp, scale=0.5)
                r0 = b * TPB + h * 128
                if split_y:
                    # nothing follows in the scalar queue at the very
                    # end, so its HWDGE can take half the store
                    nc.sync.dma_start(y_ap[r0:r0 + 128, 0:256],
                                      outt[:, 0:256])
                    nc.scalar.dma_start(y_ap[r0:r0 + 128, 256:D],
                                        outt[:, 256:D])
                else:
                    nc.sync.dma_start(y_ap[r0:r0 + 128, :], outt[:])

            def phase2(b, split_y=False):
                for h in range(2):
                    ph2_half(b, h, split_y=split_y)

            # ------- schedule: proj lookahead + pipelined A2A/phase2 -------
            xload(0)
            xload(1)
            proj(0)
            for b in range(B):
                if b + 2 < B:
                    xload(b + 2)
                if b + 1 < B:
                    proj(b + 1)
                # phase2(b-1) is split across attention(b): half 0 (whose
                # exchange finished during attention(b-1) half B) at the
                # half boundary, half 1 (exchanged only at the end of
                # attention(b-1)) late in half B so a skewed peer's A2A
                # never blocks the in-order PE queue
                cb0 = ((lambda bb: lambda: ph2_half(bb, 0))(b - 1)
                       if b >= 1 else None)
                cb1 = ((lambda bb: lambda: ph2_half(bb, 1))(b - 1)
                       if b >= 1 else None)
                attention(b, mid_cb=cb0, late_cb=cb1)
            phase2(B - 1, split_y=True)

    nc.compile()
    return nc


def _prep_inputs(x, Wq, bq, Wk, bk, Wv, bv, Wo, bo):
    xT = np.zeros((DPAD, BN), dtype=np.float32)
    xT[:D, :] = np.ascontiguousarray(x.reshape(BN, D).T)
    xT[D, :] = 1.0
    x8 = xT.astype(F8)

    woT = np.zeros((DPAD, D - 1), dtype=np.float32)
    woT[:D + 1, :] = np.concatenate([Wo.T, bo[None, :]], axis=0)
    woTb = woT.astype(BF)

    in_maps = []
    for h in range(NCORES):
        wqk = np.zeros((768, 128), dtype=np.float32)
        wqk[:D + 1, 0:64] = W8 * np.concatenate([Wq[h].T, bq[h][None, :]],
                                                axis=0)
        # negated k: folds the Lorentz score sign into the exp scale
        wqk[:D + 1, 64:128] = -W8 * np.concatenate([Wk[h].T, bk[h][None, :]],
                                                   axis=0)
        wv = np.zeros((768, DHS), dtype=np.float32)
        wv[:D + 1, :] = W8 * np.concatenate([Wv[h].T, bv[h][None, :]],
                                            axis=0)
        # pre-pair the DoubleRow k-tiles: [768, M] -> [128, 3, 2, M]
        wqk8 = np.ascontiguousarray(
            wqk.reshape(3, 2, 128, 128).transpose(2, 0, 1, 3)).astype(F8)
        wv8 = np.ascontiguousarray(
            wv.reshape(3, 2, 128, DHS).transpose(2, 0, 1, 3)).astype(F8)
        in_maps.append({
            "x8": x8,
            "wqk8": wqk8,
            "wv8": wv8,
            "woT": woTb,
        })
    return in_maps


def _run(inputs, trace=False, **kw):
    if "nc" not in _CACHE:
        _CACHE["nc"] = _build()
    nc = _CACHE["nc"]
    in_maps = _prep_inputs(**{k: np.asarray(v) for k, v in inputs.items()})
    res = bass_utils.run_bass_kernel_spmd(
        nc, in_maps, core_ids=list(range(NCORES)), trace=trace, **kw
    )
    y = np.stack([res.results[c]["y"] for c in range(NCORES)], axis=0)
    # y[c, b*256 + h*128 + i, :] holds token b*2048 + h*1024 + c*128 + i
    y = y.reshape(NCORES, B, 2, HTOK, D).transpose(1, 2, 0, 3, 4)
    return np.ascontiguousarray(y.reshape(B, N, D)), res


def kernel(**inputs):
    y, _ = _run(inputs)
    return y
